# revision 19
# baseline (speedup 1.0000x reference)
"""MiniMax Text01 Lightning Attention — 8-core Trainium2 Bass kernel (v3).

Sharding: token-sharded (data parallel over B*S). Each core handles 1024
contiguous tokens of the flattened (B*S) axis = 4 blocks of 256 for one batch.
The chunk-scan dependency across shards is resolved with a per-(batch,head)
kv "contribution" AllGather within each batch's 4-core group plus a post-hoc
correction term (o += (q*qdec*bd^i) @ kv_start).

v3 layout/scheduling changes vs the parked baseline:
  - qT/kT/v/gate stay SBUF-resident (no DRAM parking round trips)
  - attention runs chunk-major over 8-head groups (8 independent kv chains
    keep the PE fed through the scan's serial dependency)
  - o is kept TRANSPOSED [e, h, t] so the norm/gate phase needs no PE
    transposes and feeds the out-projection's stationary operand directly
  - the gate projection is issued right after the second AllGather so the
    collective flight time is hidden behind its matmuls
  - RMSNorm variance via ones-vector matmul reduction over squared o
  - elementwise work split across DVE / Act / Pool engines
"""

from contextlib import ExitStack

import ml_dtypes
import numpy as np

import concourse.bacc as bacc
import concourse.bass as bass
import concourse.mybir as mybir
import concourse.tile as tile
from concourse.bass_utils import run_bass_kernel_spmd
from concourse.masks import make_identity

F32 = mybir.dt.float32
BF16 = mybir.dt.bfloat16
AF = mybir.ActivationFunctionType
OP = mybir.AluOpType

B, S, H = 2, 4096, 2048
NH, HD = 16, 128
BLOCK = 256
EPS = 1e-5
N_CORES = 8
SHARD = (B * S) // N_CORES      # 1024 tokens/core
TT = SHARD // 128               # 8 token tiles of 128
NCH = SHARD // BLOCK            # 4 local chunks
KC = H // 128                   # 16 contraction chunks
GRP = N_CORES // B              # 4 cores per batch group
NHG = 8                         # heads per collective group


def _bcast_ap(src_1d: bass.AP, parts: int = 128) -> bass.AP:
    """Partition-broadcast a 1-D AP (for DMA replication)."""
    return bass.AP(tensor=src_1d.tensor, offset=src_1d.offset,
                   ap=[[0, parts]] + list(src_1d.ap))


def _rep_free(src: bass.AP, times: int) -> bass.AP:
    """Append a step-0 middle dim: [P, N] -> [P, times, N]."""
    return bass.AP(tensor=src.tensor, offset=src.offset,
                   ap=[src.ap[0], [0, times], src.ap[1]])


def _build():
    nc = bacc.Bacc("TRN2", target_bir_lowering=False, debug=False,
                   num_devices=N_CORES)

    x = nc.dram_tensor("x", [SHARD, H], BF16, kind="ExternalInput").ap()
    w_qkv = nc.dram_tensor("w_qkv", [H, 3 * H], BF16,
                           kind="ExternalInput").ap()
    w_gate = nc.dram_tensor("w_gate", [H, H], BF16,
                            kind="ExternalInput").ap()
    w_out = nc.dram_tensor("w_out", [H, H], BF16,
                           kind="ExternalInput").ap()
    nw = nc.dram_tensor("nw", [H], F32, kind="ExternalInput").ap()
    mask = nc.dram_tensor("mask", [SHARD], F32, kind="ExternalInput").ap()
    qdec = nc.dram_tensor("qdec", [NH, BLOCK], BF16,
                          kind="ExternalInput").ap()
    kdec = nc.dram_tensor("kdec", [NH, BLOCK], F32, kind="ExternalInput").ap()
    diagT = nc.dram_tensor("diagT", [NH, BLOCK, BLOCK], BF16,
                           kind="ExternalInput").ap()
    bd = nc.dram_tensor("bd", [NH], F32, kind="ExternalInput").ap()
    wj = nc.dram_tensor("wj", [NH, GRP], F32, kind="ExternalInput").ap()
    bdp = nc.dram_tensor("bdp", [NH, NCH], F32, kind="ExternalInput").ap()
    y = nc.dram_tensor("y", [SHARD, H], F32, kind="ExternalOutput").ap()

    with tile.TileContext(nc) as tc, ExitStack() as stack:
        # ---------------- constants (live whole kernel, left) ----------
        consts = stack.enter_context(tc.tile_pool(name="consts", bufs=1))
        # dummy operand for HAM warmup / keep-warm filler chains: the PE
        # runs at 1.2GHz until it sees ~4us of wait-free matmuls, and
        # re-throttles after ~3.4us idle (transposes don't count as busy).
        warm = consts.tile([128, 512], BF16)
        nc.vector.memset(warm, 0.001)
        ident_b = consts.tile([128, 128], BF16)
        make_identity(nc, ident_b)
        nw_sb = consts.tile([128, NH], F32)
        nc.sync.dma_start(nw_sb, nw.rearrange("(h p) -> p h", p=128))
        mask_sb = consts.tile([128, TT], F32)
        nc.sync.dma_start(mask_sb, mask.rearrange("(a p) -> p a", p=128))
        bd_sb = consts.tile([128, NH], F32)
        nc.sync.dma_start(bd_sb, _bcast_ap(bd))
        wj_sb = consts.tile([128, NH, GRP], F32)
        nc.sync.dma_start(wj_sb, _bcast_ap(wj.rearrange("h j -> (h j)")
                                           ).rearrange("p (h j) -> p h j",
                                                       h=NH))
        bdp_sb = consts.tile([128, NH, NCH], F32)
        nc.sync.dma_start(bdp_sb, _bcast_ap(bdp.rearrange("h i -> (h i)")
                                            ).rearrange("p (h i) -> p h i",
                                                        h=NH))
        kdec_sb = consts.tile([128, NH, 2], F32)
        nc.sync.dma_start(kdec_sb, kdec.rearrange("h (c p) -> p h c", p=128))
        ones_mat = consts.tile([128, 128], BF16)
        nc.vector.memset(ones_mat, 1.0)
        eps_sb = consts.tile([128, 1], F32)
        nc.vector.memset(eps_sb, EPS)
        qd_all = consts.tile([128, NH, BLOCK], BF16)
        nc.sync.dma_start(qd_all, _bcast_ap(qdec.rearrange("h m -> (h m)")
                                            ).rearrange("p (h m) -> p h m",
                                                        h=NH))

        dram = stack.enter_context(tc.tile_pool(name="dram", bufs=1,
                                                space="DRAM"))
        cc_in = dram.tile([NH, HD, HD], F32)
        cc_out = dram.tile([GRP, NHG, HD, HD], F32)
        cc_out2 = dram.tile([GRP, NHG, HD, HD], F32)

        # long-lived left-side pools. LIFO release order:
        # dg, v, kT (end of P2) -> xt, qT (end of P4)
        qT_pool = tc.alloc_tile_pool(name="qtp", bufs=1)
        qT = qT_pool.tile([128, NH, SHARD], BF16)
        xt_pool = tc.alloc_tile_pool(name="xtp", bufs=1)
        xt = xt_pool.tile([128, KC, SHARD], BF16)
        kT_pool = tc.alloc_tile_pool(name="ktp", bufs=1)
        kT = kT_pool.tile([128, NH, SHARD], BF16)
        v_pool = tc.alloc_tile_pool(name="vp", bufs=1)
        v_sb = v_pool.tile([128, TT, H], BF16)
        dg_pool = tc.alloc_tile_pool(name="dgp", bufs=1)
        dg_all = dg_pool.tile([128, NH, 2, BLOCK], BF16)

        # ---------------- P0 + P1a ----------------
        # P1a: q, k projections (W-stationary), 2-head chunks with 4
        # interleaved psum chains. First two weight chunks are DMA'd
        # before the P0 transposes so P1a starts without a stall.
        with (
            tc.tile_pool(name="wq", bufs=3) as wq_pool,
            tc.tile_pool(name="ps_mm", bufs=6, space="PSUM") as ps_mm,
        ):
            wq_pre = {}
            for chp in range(2):
                base = (0 if chp < 8 else H) + (chp % 8) * 256
                w_t = wq_pre[chp] = wq_pool.tile([128, KC, 256], BF16,
                                                 tag="wq", name=f"wqp{chp}")
                nc.sync.dma_start(
                    w_t, w_qkv[:, base:base + 256]
                    .rearrange("(kc kp) c -> kp kc c", kp=128))

            # P0: transpose x -> xt
            with (
                tc.tile_pool(name="xin", bufs=3) as xin_pool,
                tc.tile_pool(name="ps_t", bufs=2, space="PSUM") as ps_t,
            ):
                for i in range(TT):
                    x_in = xin_pool.tile([128, H], BF16, tag="xin")
                    nc.sync.dma_start(x_in, x[i * 128:(i + 1) * 128, :])
                    for kc in range(KC):
                        pst = ps_t.tile([128, 128], BF16, tag="t")
                        nc.tensor.transpose(pst,
                                            x_in[:, kc * 128:(kc + 1) * 128],
                                            ident_b)
                        dst = xt[:, kc, i * 128:(i + 1) * 128]
                        if kc % 2 == 0:
                            nc.scalar.copy(dst, pst)
                        else:
                            nc.vector.tensor_copy(dst, pst)

            for chp in range(NH):
                base = (0 if chp < 8 else H) + (chp % 8) * 256
                if chp in wq_pre:
                    w_t = wq_pre[chp]
                else:
                    w_t = wq_pool.tile([128, KC, 256], BF16, tag="wq")
                    nc.sync.dma_start(
                        w_t, w_qkv[:, base:base + 256]
                        .rearrange("(kc kp) c -> kp kc c", kp=128))
                if chp == 10:
                    # dgT tables for P2, issued late so urgent W DMAs go first
                    nc.sync.dma_start(
                        dg_all, diagT.rearrange("h (c p) m -> p h c m", p=128))
                dst = qT if chp < 8 else kT
                h0 = 2 * (chp % 8)
                ps4 = [ps_mm.tile([128, 512], F32, tag="mm",
                                  name=f"pm{chp}_{q}") for q in range(4)]
                for kc in range(KC):
                    for c2 in range(2):
                        for th in range(2):
                            nc.tensor.matmul(
                                ps4[c2 * 2 + th],
                                lhsT=w_t[:, kc, c2 * 128:(c2 + 1) * 128],
                                rhs=xt[:, kc, th * 512:(th + 1) * 512],
                                start=(kc == 0), stop=(kc == KC - 1))
                for c2 in range(2):
                    for th in range(2):
                        nc.scalar.activation(
                            dst[:, h0 + c2, th * 512:(th + 1) * 512],
                            ps4[c2 * 2 + th], AF.Silu)

        # ---------------- P1b: v projection (xt-stationary) -----------
        with (
            tc.tile_pool(name="wv", bufs=4) as wv_pool,
            tc.tile_pool(name="ps_v", bufs=8, space="PSUM") as ps_v,
        ):
            for cq in range(4):
                pv = [ps_v.tile([128, 512], F32, tag="v",
                                name=f"pv{cq}_{i}") for i in range(TT)]
                for kg in range(4):
                    wv_t = wv_pool.tile([128, 4, 512], BF16, tag="wv")
                    nc.sync.dma_start(
                        wv_t, w_qkv[kg * 512:(kg + 1) * 512,
                                    2 * H + cq * 512:2 * H + (cq + 1) * 512]
                        .rearrange("(kk kp) c -> kp kk c", kp=128))
                    for kk in range(4):
                        kc = 4 * kg + kk
                        for i in range(TT):
                            nc.tensor.matmul(
                                pv[i], lhsT=xt[:, kc, i * 128:(i + 1) * 128],
                                rhs=wv_t[:, kk], start=(kc == 0),
                                stop=(kc == KC - 1))
                for i in range(TT):
                    nc.scalar.activation(
                        v_sb[:, i, cq * 512:(cq + 1) * 512], pv[i], AF.Silu,
                        scale=mask_sb[:, i:i + 1])

        # ---------------- P2: lightning attention scan ----------------
        with (
            tc.tile_pool(name="kvsb", bufs=1) as kvsb_pool,
            tc.tile_pool(name="att", bufs=3) as att,
            tc.tile_pool(name="ps_at", bufs=2, space="PSUM") as ps_at,
            tc.tile_pool(name="ps_o", bufs=2, space="PSUM") as ps_o,
            tc.tile_pool(name="ps_kv", bufs=2, space="PSUM") as ps_kv,
            tc.tile_pool(name="ps_st", bufs=2, space="PSUM") as ps_st,
        ):
            oT_pool = tc.alloc_tile_pool(name="otp", bufs=1, side="right")
            oT = oT_pool.tile([128, NH, SHARD], BF16)
            kv_sb = kvsb_pool.tile([128, NH, HD], F32)
            kv_bf = kvsb_pool.tile([128, NH, HD], BF16)

            for g in range(2):
                for i in range(NCH):
                    iB = i * BLOCK
                    for hh in range(NHG):
                        h = g * NHG + hh
                        # intra-chunk scores^T = (k^T chunk slices) q chunk
                        at_ps = []
                        for p in range(2):
                            pa = ps_at.tile([128, BLOCK], F32, tag="at",
                                            name=f"pa{p}")
                            nc.tensor.matmul(
                                pa,
                                lhsT=kT[:, h, iB + p * 128:iB + (p + 1) * 128],
                                rhs=qT[:, h, iB:iB + BLOCK],
                                start=True, stop=True)
                            at_ps.append(pa)
                        at_sb = att.tile([128, 2, BLOCK], BF16, tag="atsb")
                        for p in range(2):
                            nc.vector.tensor_tensor(at_sb[:, p], at_ps[p],
                                                    dg_all[:, h, p], OP.mult)
                        # k natural (transposed back) scaled by kdec
                        kd = att.tile([128, 2, HD], BF16, tag="kd")
                        for p in range(2):
                            pst = ps_st.tile([128, 128], BF16, tag="st")
                            nc.tensor.transpose(
                                pst, kT[:, h, iB + p * 128:iB + (p + 1) * 128],
                                ident_b)
                            nc.scalar.activation(
                                kd[:, p], pst, AF.Copy,
                                scale=kdec_sb[:, h, p:p + 1])
                        # oT chunk = kv_state^T-contrib + intra
                        if i > 0:
                            qTd = att.tile([128, BLOCK], BF16, tag="qtd")
                            nc.gpsimd.tensor_tensor(qTd, qT[:, h, iB:iB + BLOCK],
                                                    qd_all[:, h], OP.mult)
                        po = ps_o.tile([128, BLOCK], F32, tag="o")
                        if i > 0:
                            nc.tensor.matmul(po, lhsT=kv_bf[:, h], rhs=qTd,
                                             start=True, stop=False)
                        for p in range(2):
                            nc.tensor.matmul(
                                po,
                                lhsT=v_sb[:, 2 * i + p, h * HD:(h + 1) * HD],
                                rhs=at_sb[:, p],
                                start=(i == 0 and p == 0), stop=(p == 1))
                        nc.scalar.copy(oT[:, h, iB:iB + BLOCK], po)
                        # kv state update
                        pkv = ps_kv.tile([128, HD], F32, tag="kv")
                        for p in range(2):
                            nc.tensor.matmul(
                                pkv, lhsT=kd[:, p],
                                rhs=v_sb[:, 2 * i + p, h * HD:(h + 1) * HD],
                                start=(p == 0), stop=(p == 1))
                        if i == 0:
                            nc.vector.tensor_copy(kv_sb[:, h], pkv)
                        else:
                            nc.vector.scalar_tensor_tensor(
                                kv_sb[:, h], in0=kv_sb[:, h],
                                scalar=bd_sb[:, h:h + 1], in1=pkv,
                                op0=OP.mult, op1=OP.add)
                        if i < NCH - 1:
                            nc.gpsimd.tensor_copy(kv_bf[:, h], kv_sb[:, h])
                # group done: export kv contributions, AllGather
                for hh in range(NHG):
                    h = g * NHG + hh
                    nc.sync.dma_start(cc_in[h], kv_sb[:, h])
                nc.gpsimd.collective_compute(
                    "AllGather", OP.bypass,
                    replica_groups=[[0, 1, 2, 3], [4, 5, 6, 7]],
                    ins=[cc_in[g * NHG:(g + 1) * NHG].opt()],
                    outs=[(cc_out if g == 0 else cc_out2).opt()])

        # kT, v, dgT no longer needed
        dg_pool.release()
        v_pool.release()
        kT_pool.release()

        # ---------------- P3: gate projection (fills AG2 flight) ------
        gT_pool = tc.alloc_tile_pool(name="gtp", bufs=1, side="right")
        gT = gT_pool.tile([128, KC, SHARD], BF16)
        with (
            tc.tile_pool(name="wg", bufs=4) as wg_pool,
            tc.tile_pool(name="ps_g", bufs=4, space="PSUM") as ps_g,
            tc.tile_pool(name="p4w", bufs=2, side="right") as p4w,
            tc.tile_pool(name="ps_c", bufs=2, space="PSUM") as ps_c,
        ):
            def p4_half(half, cco):
                # combine gathered kv contributions + correct o
                kvs = p4w.tile([128, NHG, HD], F32, tag="kvs",
                               name=f"kvs{half}")
                nc.vector.memset(kvs, 0.0)
                for j in range(GRP):
                    cj = p4w.tile([128, NHG, HD], F32, tag="ccj",
                                  name=f"ccj{half}{j}")
                    nc.sync.dma_start(cj, cco[j].rearrange("h p e -> p h e"))
                    tmp = p4w.tile([128, NHG, HD], F32, tag="cct",
                                   name=f"cct{half}{j}")
                    wjs = wj_sb[:, half * NHG:(half + 1) * NHG, j:j + 1]
                    wj_b = bass.AP(tensor=wjs.tensor, offset=wjs.offset,
                                   ap=[wjs.ap[0], wjs.ap[1], [0, HD]])
                    nc.gpsimd.tensor_tensor(tmp, cj, wj_b, OP.mult)
                    nc.vector.tensor_tensor(kvs, kvs, tmp, OP.add)
                kvs_bf = p4w.tile([128, NHG, HD], BF16, tag="kvsbf",
                                  name=f"kvsbf{half}")
                nc.gpsimd.tensor_copy(kvs_bf, kvs)
                for hh in range(NHG):
                    h = half * NHG + hh
                    qTdc = p4w.tile([128, NCH, BLOCK], BF16, tag="qtdc",
                                    name=f"qtdc{half}{hh}")
                    for i in range(NCH):
                        nc.vector.scalar_tensor_tensor(
                            qTdc[:, i], in0=qT[:, h, i * BLOCK:(i + 1) * BLOCK],
                            scalar=bdp_sb[:, h, i:i + 1], in1=qd_all[:, h],
                            op0=OP.mult, op1=OP.mult)
                    for m in range(2):
                        pc = ps_c.tile([128, 512], F32, tag="c",
                                       name=f"pc{half}{hh}{m}")
                        nc.tensor.matmul(
                            pc, lhsT=kvs_bf[:, hh],
                            rhs=qTdc[:, 2 * m:2 * m + 2, :],
                            start=True, stop=True)
                        osl = oT[:, h, m * 512:(m + 1) * 512]
                        nc.vector.tensor_tensor(osl, osl, pc, OP.add)

            for ch in range(KC):
                wg_t = wg_pool.tile([128, KC, 128], BF16, tag="wg")
                nc.sync.dma_start(
                    wg_t, w_gate[:, ch * 128:(ch + 1) * 128]
                    .rearrange("(kc kp) c -> kp kc c", kp=128))
                for th in range(2):
                    psg = ps_g.tile([128, 512], F32, tag="g")
                    for kc in range(KC):
                        nc.tensor.matmul(
                            psg, lhsT=wg_t[:, kc],
                            rhs=xt[:, kc, th * 512:(th + 1) * 512],
                            start=(kc == 0), stop=(kc == KC - 1))
                    nc.scalar.activation(gT[:, ch, th * 512:(th + 1) * 512],
                                         psg, AF.Sigmoid)
                if ch == 5:
                    p4_half(0, cc_out)
                elif ch == 8:
                    p4_half(1, cc_out2)

        xt_pool.release()
        qT_pool.release()

        # ---------------- P5a: RMSNorm statistics ----------------
        ogT_pool = tc.alloc_tile_pool(name="ogtp", bufs=1, side="right")
        ogT = ogT_pool.tile([128, KC, SHARD], BF16)
        rstd_pool = tc.alloc_tile_pool(name="rstdp", bufs=1, side="right")
        std_b = rstd_pool.tile([128, SHARD], F32)
        rstd_b = rstd_pool.tile([128, SHARD], F32)
        with (
            tc.tile_pool(name="sqp", bufs=2, side="right") as sq_pool,
            tc.tile_pool(name="ps_ss", bufs=2, space="PSUM") as ps_ss,
        ):
            for tcq in range(NCH):
                ts = tcq * BLOCK
                sq = sq_pool.tile([128, NH, BLOCK], BF16, tag="sq")
                nc.scalar.activation(sq[:, 0:NH // 2],
                                     oT[:, 0:NH // 2, ts:ts + BLOCK],
                                     AF.Square)
                nc.scalar.activation(sq[:, NH // 2:],
                                     oT[:, NH // 2:, ts:ts + BLOCK],
                                     AF.Square)
                # all-ones stationary: every output partition gets the
                # cross-partition sum, so the broadcast is free
                pss = ps_ss.tile([128, BLOCK], F32, tag="ss")
                for h in range(NH):
                    nc.tensor.matmul(pss, lhsT=ones_mat, rhs=sq[:, h],
                                     start=(h == 0), stop=(h == NH - 1))
                nc.scalar.activation(std_b[:, ts:ts + BLOCK], pss, AF.Sqrt,
                                     bias=eps_sb[:, 0:1], scale=1.0 / H)
            nc.vector.reciprocal(rstd_b, std_b)

        # ---------------- P5b + P6: gate/norm multiply, out proj ------
        wo_pool = tc.alloc_tile_pool(name="wop", bufs=1, side="right")
        wo_sb = wo_pool.tile([128, KC, H], BF16)
        for kg in range(4):
            nc.sync.dma_start(
                wo_sb[:, 4 * kg:4 * (kg + 1), :],
                w_out[kg * 512:(kg + 1) * 512, :]
                .rearrange("(kc kp) c -> kp kc c", kp=128))

        with (
            tc.tile_pool(name="p5w", bufs=3, side="right") as p5w,
            tc.tile_pool(name="ost", bufs=4) as ost_pool,
            tc.tile_pool(name="ps_mo", bufs=4, space="PSUM") as ps_mo,
        ):
            for tcq in range(NCH):
                ts = tcq * BLOCK
                for h in range(NH):
                    tmp = p5w.tile([128, BLOCK], F32, tag="t5")
                    nc.gpsimd.tensor_tensor(tmp, oT[:, h, ts:ts + BLOCK],
                                            rstd_b[:, ts:ts + BLOCK], OP.mult)
                    nc.vector.scalar_tensor_tensor(
                        ogT[:, h, ts:ts + BLOCK], in0=tmp,
                        scalar=nw_sb[:, h:h + 1], in1=gT[:, h, ts:ts + BLOCK],
                        op0=OP.mult, op1=OP.mult)
                for tt in range(2):
                    t0 = ts + tt * 128
                    pso = [ps_mo.tile([128, 512], F32, tag="mo",
                                      name=f"po{tcq}{tt}{j}")
                           for j in range(4)]
                    for kc in range(KC):
                        for j in range(4):
                            nc.tensor.matmul(
                                pso[j], lhsT=ogT[:, kc, t0:t0 + 128],
                                rhs=wo_sb[:, kc, j * 512:(j + 1) * 512],
                                start=(kc == 0), stop=(kc == KC - 1))
                    for j in range(4):
                        ost = ost_pool.tile([128, 512], F32, tag="ost")
                        nc.scalar.copy(ost, pso[j])
                        nc.sync.dma_start(
                            y[t0:t0 + 128, j * 512:(j + 1) * 512], ost)

        wo_pool.release()
        rstd_pool.release()
        ogT_pool.release()
        gT_pool.release()
        oT_pool.release()

    nc.compile()
    return nc


_CACHED = None


def _get_nc():
    global _CACHED
    if _CACHED is None:
        _CACHED = _build()
    return _CACHED


def _host_tables(slope: np.ndarray):
    slope = slope.astype(np.float32)
    ar = np.arange(BLOCK, dtype=np.float32) + 1.0
    qdec = np.exp(-slope[:, None] * ar[None, :]).astype(np.float32)
    kdec = np.exp(-slope[:, None] * (BLOCK - ar)[None, :]).astype(np.float32)
    idx = ar[:, None] - ar[None, :]
    m2 = (idx >= 0).astype(np.float32)
    diag = np.exp(-slope[:, None, None] * (idx * m2)[None]) * m2[None]
    diagT = np.ascontiguousarray(diag.transpose(0, 2, 1)).astype(np.float32)
    bd = np.exp(-slope * BLOCK).astype(np.float32)
    bdp = np.stack([bd ** i for i in range(NCH)], axis=1).astype(np.float32)
    return qdec, kdec, diagT, bd, bdp


def _make_in_maps(hidden_states, attention_mask, slope_rate, w_qkv, w_gate,
                  w_out, norm_weight):
    BF = ml_dtypes.bfloat16
    hs = np.ascontiguousarray(np.asarray(hidden_states, np.float32)
                              .reshape(B * S, H).astype(BF))
    mask = np.ascontiguousarray(np.asarray(attention_mask, np.float32)
                                .reshape(B * S))
    w_qkv = np.ascontiguousarray(np.asarray(w_qkv, np.float32).astype(BF))
    w_gate = np.ascontiguousarray(np.asarray(w_gate, np.float32).astype(BF))
    w_out = np.ascontiguousarray(np.asarray(w_out, np.float32).astype(BF))
    nw = np.ascontiguousarray(np.asarray(norm_weight, np.float32))
    slope = np.asarray(slope_rate, np.float32)
    qdec, kdec, diagT, bd, bdp = _host_tables(slope)
    qdec_bf = np.ascontiguousarray(qdec.astype(BF))
    diagT_bf = np.ascontiguousarray(diagT.astype(BF))

    in_maps = []
    for c in range(N_CORES):
        r = c % GRP
        wj = np.zeros((NH, GRP), np.float32)
        for j in range(r):
            wj[:, j] = bd ** (4 * (r - 1 - j))
        in_maps.append({
            "x": np.ascontiguousarray(hs[c * SHARD:(c + 1) * SHARD]),
            "mask": np.ascontiguousarray(mask[c * SHARD:(c + 1) * SHARD]),
            "w_qkv": w_qkv, "w_gate": w_gate, "w_out": w_out, "nw": nw,
            "qdec": qdec_bf, "kdec": kdec, "diagT": diagT_bf, "bd": bd,
            "wj": wj, "bdp": bdp,
        })
    return in_maps


def kernel(hidden_states, attention_mask, slope_rate, w_qkv, w_gate, w_out,
           norm_weight):
    nc = _get_nc()
    in_maps = _make_in_maps(hidden_states, attention_mask, slope_rate, w_qkv,
                            w_gate, w_out, norm_weight)

    import os
    trace = bool(int(os.environ.get("KERNEL_TRACE", "0")))
    res = run_bass_kernel_spmd(nc, in_maps, core_ids=list(range(N_CORES)),
                               trace=trace)
    kernel.last_results = res
    out = np.concatenate([res.results[c]["y"] for c in range(N_CORES)], axis=0)
    return out.reshape(B, S, H)


# revision 20
# speedup vs baseline: 1.0103x; 1.0103x over previous
"""MiniMax Text01 Lightning Attention — 8-core Trainium2 Bass kernel (v3).

Sharding: token-sharded (data parallel over B*S). Each core handles 1024
contiguous tokens of the flattened (B*S) axis = 4 blocks of 256 for one batch.
The chunk-scan dependency across shards is resolved with a per-(batch,head)
kv "contribution" AllGather within each batch's 4-core group plus a post-hoc
correction term (o += (q*qdec*bd^i) @ kv_start).

v3 layout/scheduling changes vs the parked baseline:
  - qT/kT/v/gate stay SBUF-resident (no DRAM parking round trips)
  - attention runs chunk-major over 8-head groups (8 independent kv chains
    keep the PE fed through the scan's serial dependency)
  - o is kept TRANSPOSED [e, h, t] so the norm/gate phase needs no PE
    transposes and feeds the out-projection's stationary operand directly
  - the gate projection is issued right after the second AllGather so the
    collective flight time is hidden behind its matmuls
  - RMSNorm variance via ones-vector matmul reduction over squared o
  - elementwise work split across DVE / Act / Pool engines
"""

from contextlib import ExitStack

import ml_dtypes
import numpy as np

import concourse.bacc as bacc
import concourse.bass as bass
import concourse.mybir as mybir
import concourse.tile as tile
from concourse.bass_utils import run_bass_kernel_spmd
from concourse.masks import make_identity

F32 = mybir.dt.float32
BF16 = mybir.dt.bfloat16
AF = mybir.ActivationFunctionType
OP = mybir.AluOpType

B, S, H = 2, 4096, 2048
NH, HD = 16, 128
BLOCK = 256
EPS = 1e-5
N_CORES = 8
SHARD = (B * S) // N_CORES      # 1024 tokens/core
TT = SHARD // 128               # 8 token tiles of 128
NCH = SHARD // BLOCK            # 4 local chunks
KC = H // 128                   # 16 contraction chunks
GRP = N_CORES // B              # 4 cores per batch group
NHG = 8                         # heads per collective group


def _bcast_ap(src_1d: bass.AP, parts: int = 128) -> bass.AP:
    """Partition-broadcast a 1-D AP (for DMA replication)."""
    return bass.AP(tensor=src_1d.tensor, offset=src_1d.offset,
                   ap=[[0, parts]] + list(src_1d.ap))


def _rep_free(src: bass.AP, times: int) -> bass.AP:
    """Append a step-0 middle dim: [P, N] -> [P, times, N]."""
    return bass.AP(tensor=src.tensor, offset=src.offset,
                   ap=[src.ap[0], [0, times], src.ap[1]])


def _build():
    nc = bacc.Bacc("TRN2", target_bir_lowering=False, debug=False,
                   num_devices=N_CORES)

    x = nc.dram_tensor("x", [SHARD, H], BF16, kind="ExternalInput").ap()
    w_qkv = nc.dram_tensor("w_qkv", [H, 3 * H], BF16,
                           kind="ExternalInput").ap()
    w_gate = nc.dram_tensor("w_gate", [H, H], BF16,
                            kind="ExternalInput").ap()
    w_out = nc.dram_tensor("w_out", [H, H], BF16,
                           kind="ExternalInput").ap()
    nw = nc.dram_tensor("nw", [H], F32, kind="ExternalInput").ap()
    mask = nc.dram_tensor("mask", [SHARD], F32, kind="ExternalInput").ap()
    qdec = nc.dram_tensor("qdec", [NH, BLOCK], BF16,
                          kind="ExternalInput").ap()
    kdec = nc.dram_tensor("kdec", [NH, BLOCK], F32, kind="ExternalInput").ap()
    diagT = nc.dram_tensor("diagT", [NH, BLOCK, BLOCK], BF16,
                           kind="ExternalInput").ap()
    bd = nc.dram_tensor("bd", [NH], F32, kind="ExternalInput").ap()
    wj = nc.dram_tensor("wj", [NH, GRP], F32, kind="ExternalInput").ap()
    bdp = nc.dram_tensor("bdp", [NH, NCH], F32, kind="ExternalInput").ap()
    y = nc.dram_tensor("y", [SHARD, H], F32, kind="ExternalOutput").ap()

    with tile.TileContext(nc) as tc, ExitStack() as stack:
        # ---------------- constants (live whole kernel, left) ----------
        consts = stack.enter_context(tc.tile_pool(name="consts", bufs=1))
        # dummy operand for HAM warmup / keep-warm filler chains: the PE
        # runs at 1.2GHz until it sees ~4us of wait-free matmuls, and
        # re-throttles after ~3.4us idle (transposes don't count as busy).
        warm = consts.tile([128, 512], BF16)
        nc.vector.memset(warm, 0.001)
        ident_b = consts.tile([128, 128], BF16)
        make_identity(nc, ident_b)
        nw_sb = consts.tile([128, NH], F32)
        nc.sync.dma_start(nw_sb, nw.rearrange("(h p) -> p h", p=128))
        mask_sb = consts.tile([128, TT], F32)
        nc.sync.dma_start(mask_sb, mask.rearrange("(a p) -> p a", p=128))
        bd_sb = consts.tile([128, NH], F32)
        nc.sync.dma_start(bd_sb, _bcast_ap(bd))
        wj_sb = consts.tile([128, NH, GRP], F32)
        nc.sync.dma_start(wj_sb, _bcast_ap(wj.rearrange("h j -> (h j)")
                                           ).rearrange("p (h j) -> p h j",
                                                       h=NH))
        bdp_sb = consts.tile([128, NH, NCH], F32)
        nc.sync.dma_start(bdp_sb, _bcast_ap(bdp.rearrange("h i -> (h i)")
                                            ).rearrange("p (h i) -> p h i",
                                                        h=NH))
        kdec_sb = consts.tile([128, NH, 2], F32)
        nc.sync.dma_start(kdec_sb, kdec.rearrange("h (c p) -> p h c", p=128))
        ones_mat = consts.tile([128, 128], BF16)
        nc.vector.memset(ones_mat, 1.0)
        eps_sb = consts.tile([128, 1], F32)
        nc.vector.memset(eps_sb, EPS)
        qd_all = consts.tile([128, NH, BLOCK], BF16)
        nc.sync.dma_start(qd_all, _bcast_ap(qdec.rearrange("h m -> (h m)")
                                            ).rearrange("p (h m) -> p h m",
                                                        h=NH))

        dram = stack.enter_context(tc.tile_pool(name="dram", bufs=1,
                                                space="DRAM"))
        cc_in = dram.tile([NH, HD, HD], F32)
        cc_out = dram.tile([GRP, NHG, HD, HD], F32)
        cc_out2 = dram.tile([GRP, NHG, HD, HD], F32)

        # long-lived left-side pools. LIFO release order:
        # dg, v, kT (end of P2) -> xt, qT (end of P4)
        qT_pool = tc.alloc_tile_pool(name="qtp", bufs=1)
        qT = qT_pool.tile([128, NH, SHARD], BF16)
        xt_pool = tc.alloc_tile_pool(name="xtp", bufs=1)
        xt = xt_pool.tile([128, KC, SHARD], BF16)
        kT_pool = tc.alloc_tile_pool(name="ktp", bufs=1)
        kT = kT_pool.tile([128, NH, SHARD], BF16)
        v_pool = tc.alloc_tile_pool(name="vp", bufs=1)
        v_sb = v_pool.tile([128, TT, H], BF16)
        dg_pool = tc.alloc_tile_pool(name="dgp", bufs=1)
        dg_all = dg_pool.tile([128, NH, 2, BLOCK], BF16)

        # ---------------- P0: transpose x -> xt ----------------
        with (
            tc.tile_pool(name="xin", bufs=2) as xin_pool,
            tc.tile_pool(name="ps_t", bufs=3, space="PSUM") as ps_t,
        ):
            for i in range(TT):
                x_in = xin_pool.tile([128, H], BF16, tag="xin")
                nc.sync.dma_start(x_in, x[i * 128:(i + 1) * 128, :])
                for kc in range(KC):
                    pst = ps_t.tile([128, 128], BF16, tag="t")
                    nc.tensor.transpose(pst, x_in[:, kc * 128:(kc + 1) * 128],
                                        ident_b)
                    dst = xt[:, kc, i * 128:(i + 1) * 128]
                    if kc % 2 == 0:
                        nc.scalar.copy(dst, pst)
                    else:
                        nc.vector.tensor_copy(dst, pst)

        # ---------------- P1a: q, k projections (W-stationary) --------
        # 2-head chunks with 4 interleaved psum chains: sem waits only
        # every 64 matmuls keeps the PE stream dense.
        with (
            tc.tile_pool(name="wq", bufs=3) as wq_pool,
            tc.tile_pool(name="ps_mm", bufs=8, space="PSUM") as ps_mm,
        ):
            for chp in range(NH):
                base = (0 if chp < 8 else H) + (chp % 8) * 256
                w_t = wq_pool.tile([128, KC, 256], BF16, tag="wq")
                nc.sync.dma_start(
                    w_t, w_qkv[:, base:base + 256]
                    .rearrange("(kc kp) c -> kp kc c", kp=128))
                if chp == 10:
                    # dgT tables for P2, issued late so urgent W DMAs go first
                    nc.sync.dma_start(
                        dg_all, diagT.rearrange("h (c p) m -> p h c m", p=128))
                dst = qT if chp < 8 else kT
                h0 = 2 * (chp % 8)
                ps4 = [ps_mm.tile([128, 512], F32, tag="mm",
                                  name=f"pm{chp}_{q}") for q in range(4)]
                for kc in range(KC):
                    for c2 in range(2):
                        for th in range(2):
                            nc.tensor.matmul(
                                ps4[c2 * 2 + th],
                                lhsT=w_t[:, kc, c2 * 128:(c2 + 1) * 128],
                                rhs=xt[:, kc, th * 512:(th + 1) * 512],
                                start=(kc == 0), stop=(kc == KC - 1))
                for c2 in range(2):
                    for th in range(2):
                        nc.scalar.activation(
                            dst[:, h0 + c2, th * 512:(th + 1) * 512],
                            ps4[c2 * 2 + th], AF.Silu)

        # ---------------- P1b: v projection (xt-stationary) -----------
        with (
            tc.tile_pool(name="wv", bufs=4) as wv_pool,
            tc.tile_pool(name="ps_v", bufs=8, space="PSUM") as ps_v,
        ):
            for cq in range(4):
                pv = [ps_v.tile([128, 512], F32, tag="v",
                                name=f"pv{cq}_{i}") for i in range(TT)]
                for kg in range(4):
                    wv_t = wv_pool.tile([128, 4, 512], BF16, tag="wv")
                    nc.sync.dma_start(
                        wv_t, w_qkv[kg * 512:(kg + 1) * 512,
                                    2 * H + cq * 512:2 * H + (cq + 1) * 512]
                        .rearrange("(kk kp) c -> kp kk c", kp=128))
                    for kk in range(4):
                        kc = 4 * kg + kk
                        for i in range(TT):
                            nc.tensor.matmul(
                                pv[i], lhsT=xt[:, kc, i * 128:(i + 1) * 128],
                                rhs=wv_t[:, kk], start=(kc == 0),
                                stop=(kc == KC - 1))
                for i in range(TT):
                    nc.scalar.activation(
                        v_sb[:, i, cq * 512:(cq + 1) * 512], pv[i], AF.Silu,
                        scale=mask_sb[:, i:i + 1])

        # ---------------- P2: lightning attention scan ----------------
        with (
            tc.tile_pool(name="kvsb", bufs=1) as kvsb_pool,
            tc.tile_pool(name="att", bufs=3) as att,
            tc.tile_pool(name="ps_at", bufs=2, space="PSUM") as ps_at,
            tc.tile_pool(name="ps_o", bufs=2, space="PSUM") as ps_o,
            tc.tile_pool(name="ps_kv", bufs=2, space="PSUM") as ps_kv,
            tc.tile_pool(name="ps_st", bufs=2, space="PSUM") as ps_st,
        ):
            oT_pool = tc.alloc_tile_pool(name="otp", bufs=1, side="right")
            oT = oT_pool.tile([128, NH, SHARD], BF16)
            kv_sb = kvsb_pool.tile([128, NH, HD], F32)
            kv_bf = kvsb_pool.tile([128, NH, HD], BF16)

            for g in range(2):
                for i in range(NCH):
                    iB = i * BLOCK
                    for hh in range(NHG):
                        h = g * NHG + hh
                        # intra-chunk scores^T = (k^T chunk slices) q chunk
                        at_ps = []
                        for p in range(2):
                            pa = ps_at.tile([128, BLOCK], F32, tag="at",
                                            name=f"pa{p}")
                            nc.tensor.matmul(
                                pa,
                                lhsT=kT[:, h, iB + p * 128:iB + (p + 1) * 128],
                                rhs=qT[:, h, iB:iB + BLOCK],
                                start=True, stop=True)
                            at_ps.append(pa)
                        at_sb = att.tile([128, 2, BLOCK], BF16, tag="atsb")
                        for p in range(2):
                            nc.vector.tensor_tensor(at_sb[:, p], at_ps[p],
                                                    dg_all[:, h, p], OP.mult)
                        # k natural (transposed back) scaled by kdec
                        kd = att.tile([128, 2, HD], BF16, tag="kd")
                        for p in range(2):
                            pst = ps_st.tile([128, 128], BF16, tag="st")
                            nc.tensor.transpose(
                                pst, kT[:, h, iB + p * 128:iB + (p + 1) * 128],
                                ident_b)
                            nc.scalar.activation(
                                kd[:, p], pst, AF.Copy,
                                scale=kdec_sb[:, h, p:p + 1])
                        # oT chunk = kv_state^T-contrib + intra
                        if i > 0:
                            qTd = att.tile([128, BLOCK], BF16, tag="qtd")
                            nc.gpsimd.tensor_tensor(qTd, qT[:, h, iB:iB + BLOCK],
                                                    qd_all[:, h], OP.mult)
                        po = ps_o.tile([128, BLOCK], F32, tag="o")
                        if i > 0:
                            nc.tensor.matmul(po, lhsT=kv_bf[:, h], rhs=qTd,
                                             start=True, stop=False)
                        for p in range(2):
                            nc.tensor.matmul(
                                po,
                                lhsT=v_sb[:, 2 * i + p, h * HD:(h + 1) * HD],
                                rhs=at_sb[:, p],
                                start=(i == 0 and p == 0), stop=(p == 1))
                        nc.scalar.copy(oT[:, h, iB:iB + BLOCK], po)
                        # kv state update
                        pkv = ps_kv.tile([128, HD], F32, tag="kv")
                        for p in range(2):
                            nc.tensor.matmul(
                                pkv, lhsT=kd[:, p],
                                rhs=v_sb[:, 2 * i + p, h * HD:(h + 1) * HD],
                                start=(p == 0), stop=(p == 1))
                        if i == 0:
                            nc.vector.tensor_copy(kv_sb[:, h], pkv)
                        else:
                            nc.vector.scalar_tensor_tensor(
                                kv_sb[:, h], in0=kv_sb[:, h],
                                scalar=bd_sb[:, h:h + 1], in1=pkv,
                                op0=OP.mult, op1=OP.add)
                        if i < NCH - 1:
                            nc.gpsimd.tensor_copy(kv_bf[:, h], kv_sb[:, h])
                # group done: export kv contributions, AllGather
                for hh in range(NHG):
                    h = g * NHG + hh
                    nc.sync.dma_start(cc_in[h], kv_sb[:, h])
                nc.gpsimd.collective_compute(
                    "AllGather", OP.bypass,
                    replica_groups=[[0, 1, 2, 3], [4, 5, 6, 7]],
                    ins=[cc_in[g * NHG:(g + 1) * NHG].opt()],
                    outs=[(cc_out if g == 0 else cc_out2).opt()])

        # kT, v, dgT no longer needed
        dg_pool.release()
        v_pool.release()
        kT_pool.release()

        # ---------------- P3: gate projection (fills AG2 flight) ------
        gT_pool = tc.alloc_tile_pool(name="gtp", bufs=1, side="right")
        gT = gT_pool.tile([128, KC, SHARD], BF16)
        with (
            tc.tile_pool(name="wg", bufs=4) as wg_pool,
            tc.tile_pool(name="ps_g", bufs=4, space="PSUM") as ps_g,
            tc.tile_pool(name="p4w", bufs=2, side="right") as p4w,
            tc.tile_pool(name="ps_c", bufs=2, space="PSUM") as ps_c,
        ):
            def p4_half(half, cco):
                # combine gathered kv contributions + correct o
                kvs = p4w.tile([128, NHG, HD], F32, tag="kvs",
                               name=f"kvs{half}")
                nc.vector.memset(kvs, 0.0)
                for j in range(GRP):
                    cj = p4w.tile([128, NHG, HD], F32, tag="ccj",
                                  name=f"ccj{half}{j}")
                    nc.sync.dma_start(cj, cco[j].rearrange("h p e -> p h e"))
                    tmp = p4w.tile([128, NHG, HD], F32, tag="cct",
                                   name=f"cct{half}{j}")
                    wjs = wj_sb[:, half * NHG:(half + 1) * NHG, j:j + 1]
                    wj_b = bass.AP(tensor=wjs.tensor, offset=wjs.offset,
                                   ap=[wjs.ap[0], wjs.ap[1], [0, HD]])
                    nc.gpsimd.tensor_tensor(tmp, cj, wj_b, OP.mult)
                    nc.vector.tensor_tensor(kvs, kvs, tmp, OP.add)
                kvs_bf = p4w.tile([128, NHG, HD], BF16, tag="kvsbf",
                                  name=f"kvsbf{half}")
                nc.gpsimd.tensor_copy(kvs_bf, kvs)
                for hh in range(NHG):
                    h = half * NHG + hh
                    qTdc = p4w.tile([128, NCH, BLOCK], BF16, tag="qtdc",
                                    name=f"qtdc{half}{hh}")
                    for i in range(NCH):
                        nc.vector.scalar_tensor_tensor(
                            qTdc[:, i], in0=qT[:, h, i * BLOCK:(i + 1) * BLOCK],
                            scalar=bdp_sb[:, h, i:i + 1], in1=qd_all[:, h],
                            op0=OP.mult, op1=OP.mult)
                    for m in range(2):
                        pc = ps_c.tile([128, 512], F32, tag="c",
                                       name=f"pc{half}{hh}{m}")
                        nc.tensor.matmul(
                            pc, lhsT=kvs_bf[:, hh],
                            rhs=qTdc[:, 2 * m:2 * m + 2, :],
                            start=True, stop=True)
                        osl = oT[:, h, m * 512:(m + 1) * 512]
                        nc.vector.tensor_tensor(osl, osl, pc, OP.add)

            for ch in range(KC):
                wg_t = wg_pool.tile([128, KC, 128], BF16, tag="wg")
                nc.sync.dma_start(
                    wg_t, w_gate[:, ch * 128:(ch + 1) * 128]
                    .rearrange("(kc kp) c -> kp kc c", kp=128))
                for th in range(2):
                    psg = ps_g.tile([128, 512], F32, tag="g")
                    for kc in range(KC):
                        nc.tensor.matmul(
                            psg, lhsT=wg_t[:, kc],
                            rhs=xt[:, kc, th * 512:(th + 1) * 512],
                            start=(kc == 0), stop=(kc == KC - 1))
                    nc.scalar.activation(gT[:, ch, th * 512:(th + 1) * 512],
                                         psg, AF.Sigmoid)
                if ch == 6:
                    p4_half(0, cc_out)
                elif ch == 10:
                    p4_half(1, cc_out2)

        xt_pool.release()
        qT_pool.release()

        # ---------------- P5a: RMSNorm statistics ----------------
        ogT_pool = tc.alloc_tile_pool(name="ogtp", bufs=1, side="right")
        ogT = ogT_pool.tile([128, KC, SHARD], BF16)
        rstd_pool = tc.alloc_tile_pool(name="rstdp", bufs=1, side="right")
        std_b = rstd_pool.tile([128, SHARD], F32)
        rstd_b = rstd_pool.tile([128, SHARD], F32)
        with (
            tc.tile_pool(name="sqp", bufs=2, side="right") as sq_pool,
            tc.tile_pool(name="ps_ss", bufs=2, space="PSUM") as ps_ss,
        ):
            for tcq in range(NCH):
                ts = tcq * BLOCK
                sq = sq_pool.tile([128, NH, BLOCK], BF16, tag="sq")
                nc.scalar.activation(sq[:, 0:NH // 2],
                                     oT[:, 0:NH // 2, ts:ts + BLOCK],
                                     AF.Square)
                nc.scalar.activation(sq[:, NH // 2:],
                                     oT[:, NH // 2:, ts:ts + BLOCK],
                                     AF.Square)
                # all-ones stationary: every output partition gets the
                # cross-partition sum, so the broadcast is free
                pss = ps_ss.tile([128, BLOCK], F32, tag="ss")
                for h in range(NH):
                    nc.tensor.matmul(pss, lhsT=ones_mat, rhs=sq[:, h],
                                     start=(h == 0), stop=(h == NH - 1))
                nc.scalar.activation(std_b[:, ts:ts + BLOCK], pss, AF.Sqrt,
                                     bias=eps_sb[:, 0:1], scale=1.0 / H)
            nc.vector.reciprocal(rstd_b, std_b)

        # ---------------- P5b + P6: gate/norm multiply, out proj ------
        wo_pool = tc.alloc_tile_pool(name="wop", bufs=1, side="right")
        wo_sb = wo_pool.tile([128, KC, H], BF16)
        for kg in range(4):
            nc.sync.dma_start(
                wo_sb[:, 4 * kg:4 * (kg + 1), :],
                w_out[kg * 512:(kg + 1) * 512, :]
                .rearrange("(kc kp) c -> kp kc c", kp=128))

        with (
            tc.tile_pool(name="p5w", bufs=3, side="right") as p5w,
            tc.tile_pool(name="ost", bufs=4) as ost_pool,
            tc.tile_pool(name="ps_mo", bufs=4, space="PSUM") as ps_mo,
        ):
            for tcq in range(NCH):
                ts = tcq * BLOCK
                for h in range(NH):
                    tmp = p5w.tile([128, BLOCK], F32, tag="t5")
                    nc.gpsimd.tensor_tensor(tmp, oT[:, h, ts:ts + BLOCK],
                                            rstd_b[:, ts:ts + BLOCK], OP.mult)
                    nc.vector.scalar_tensor_tensor(
                        ogT[:, h, ts:ts + BLOCK], in0=tmp,
                        scalar=nw_sb[:, h:h + 1], in1=gT[:, h, ts:ts + BLOCK],
                        op0=OP.mult, op1=OP.mult)
                for tt in range(2):
                    t0 = ts + tt * 128
                    pso = [ps_mo.tile([128, 512], F32, tag="mo",
                                      name=f"po{tcq}{tt}{j}")
                           for j in range(4)]
                    for kc in range(KC):
                        for j in range(4):
                            nc.tensor.matmul(
                                pso[j], lhsT=ogT[:, kc, t0:t0 + 128],
                                rhs=wo_sb[:, kc, j * 512:(j + 1) * 512],
                                start=(kc == 0), stop=(kc == KC - 1))
                    for j in range(4):
                        ost = ost_pool.tile([128, 512], F32, tag="ost")
                        nc.scalar.copy(ost, pso[j])
                        nc.sync.dma_start(
                            y[t0:t0 + 128, j * 512:(j + 1) * 512], ost)

        wo_pool.release()
        rstd_pool.release()
        ogT_pool.release()
        gT_pool.release()
        oT_pool.release()

    nc.compile()
    return nc


_CACHED = None


def _get_nc():
    global _CACHED
    if _CACHED is None:
        _CACHED = _build()
    return _CACHED


def _host_tables(slope: np.ndarray):
    slope = slope.astype(np.float32)
    ar = np.arange(BLOCK, dtype=np.float32) + 1.0
    qdec = np.exp(-slope[:, None] * ar[None, :]).astype(np.float32)
    kdec = np.exp(-slope[:, None] * (BLOCK - ar)[None, :]).astype(np.float32)
    idx = ar[:, None] - ar[None, :]
    m2 = (idx >= 0).astype(np.float32)
    diag = np.exp(-slope[:, None, None] * (idx * m2)[None]) * m2[None]
    diagT = np.ascontiguousarray(diag.transpose(0, 2, 1)).astype(np.float32)
    bd = np.exp(-slope * BLOCK).astype(np.float32)
    bdp = np.stack([bd ** i for i in range(NCH)], axis=1).astype(np.float32)
    return qdec, kdec, diagT, bd, bdp


def _make_in_maps(hidden_states, attention_mask, slope_rate, w_qkv, w_gate,
                  w_out, norm_weight):
    BF = ml_dtypes.bfloat16
    hs = np.ascontiguousarray(np.asarray(hidden_states, np.float32)
                              .reshape(B * S, H).astype(BF))
    mask = np.ascontiguousarray(np.asarray(attention_mask, np.float32)
                                .reshape(B * S))
    w_qkv = np.ascontiguousarray(np.asarray(w_qkv, np.float32).astype(BF))
    w_gate = np.ascontiguousarray(np.asarray(w_gate, np.float32).astype(BF))
    w_out = np.ascontiguousarray(np.asarray(w_out, np.float32).astype(BF))
    nw = np.ascontiguousarray(np.asarray(norm_weight, np.float32))
    slope = np.asarray(slope_rate, np.float32)
    qdec, kdec, diagT, bd, bdp = _host_tables(slope)
    qdec_bf = np.ascontiguousarray(qdec.astype(BF))
    diagT_bf = np.ascontiguousarray(diagT.astype(BF))

    in_maps = []
    for c in range(N_CORES):
        r = c % GRP
        wj = np.zeros((NH, GRP), np.float32)
        for j in range(r):
            wj[:, j] = bd ** (4 * (r - 1 - j))
        in_maps.append({
            "x": np.ascontiguousarray(hs[c * SHARD:(c + 1) * SHARD]),
            "mask": np.ascontiguousarray(mask[c * SHARD:(c + 1) * SHARD]),
            "w_qkv": w_qkv, "w_gate": w_gate, "w_out": w_out, "nw": nw,
            "qdec": qdec_bf, "kdec": kdec, "diagT": diagT_bf, "bd": bd,
            "wj": wj, "bdp": bdp,
        })
    return in_maps


def kernel(hidden_states, attention_mask, slope_rate, w_qkv, w_gate, w_out,
           norm_weight):
    nc = _get_nc()
    in_maps = _make_in_maps(hidden_states, attention_mask, slope_rate, w_qkv,
                            w_gate, w_out, norm_weight)

    import os
    trace = bool(int(os.environ.get("KERNEL_TRACE", "0")))
    res = run_bass_kernel_spmd(nc, in_maps, core_ids=list(range(N_CORES)),
                               trace=trace)
    kernel.last_results = res
    out = np.concatenate([res.results[c]["y"] for c in range(N_CORES)], axis=0)
    return out.reshape(B, S, H)
